# revision 4
# baseline (speedup 1.0000x reference)
"""
Trainium2 Bass kernel for nn_LoraQuantLinear (NF4 quantized linear + LoRA).

Host folds dequant + absmax + LoRA into an fp16 effective weight matrix,
pre-transposed per core; the device runs a memory-bound fp16 GEMM:
  out[t, o] = sum_i xT[i, t] * wT[i, o]
Weight DMAs alternate between the two HWDGE rings (sync/scalar); a PE
warm-up burst ramps the clock before real matmuls arrive, and output
quarters are copied/stored as soon as their accumulation stops.
Sharding: out_features split across 8 cores; per-core output shards
concatenated on the feature axis.
"""

import sys

sys.path.insert(0, "/opt/trn_rl_repo")

import numpy as np

import concourse.bass as bass  # noqa: F401
import concourse.tile as tile
from concourse import bacc, mybir
from concourse.bass_utils import run_bass_kernel_spmd

TOK = 64
IN = 4096
OUT = 14336
R = 16
BLOCK = 64
LORA_SCALING = 2.0
N_CORES = 8
O_SHARD = OUT // N_CORES        # 1792
K_CHUNKS = IN // 128            # 32
N_BLOCKS = IN // BLOCK          # 64
OQ = O_SHARD // 4               # 448 psum quarter
GROUPS = [4, 4, 4, 4, 4, 4, 4, 4]      # k-chunks per weight DMA
assert sum(GROUPS) == K_CHUNKS

NF4 = np.array([
    -1.0, -0.6961928009986877, -0.5250730514526367, -0.39491748809814453,
    -0.28444138169288635, -0.18477343022823334, -0.09105003625154495, 0.0,
    0.07958029955625534, 0.16093020141124725, 0.24611230194568634, 0.33791524171829224,
    0.44070982933044434, 0.5626170039176941, 0.7229568362236023, 1.0,
], dtype=np.float32)

F16 = mybir.dt.float16
F32 = mybir.dt.float32

_CACHE = {}


def _build():
    nc = bacc.Bacc(None, target_bir_lowering=False)
    # xTp: x transposed and pre-tiled on host to [128, K_CHUNKS*TOK]
    xt_d = nc.dram_tensor("xTp", [128, K_CHUNKS * TOK], F16, kind="ExternalInput")
    # weights pre-swizzled on host, flat; group g is a [128, GROUPS[g]*O_SHARD]
    # row-major block so each partition line is one contiguous descriptor
    wt_d = nc.dram_tensor("wTs", [IN * O_SHARD], F16, kind="ExternalInput")
    out_d = nc.dram_tensor("out", [TOK, O_SHARD], F16, kind="ExternalOutput")

    with tile.TileContext(nc) as tc:
        with (
            tc.tile_pool(name="const", bufs=1) as cpool,
            tc.tile_pool(name="w", bufs=4) as wpool,
            tc.tile_pool(name="w2", bufs=2) as wpool2,
            tc.tile_pool(name="ps", bufs=1, space="PSUM") as ps,
        ):
            # x rides the sync ring while the first weight group uses scalar
            xT = cpool.tile([128, K_CHUNKS * TOK], F16)
            nc.sync.dma_start(xT[:], xt_d[:])

            pos = [ps.tile([TOK, OQ], F32, tag=f"po{q}", name=f"po{q}")
                   for q in range(4)]
            o16 = cpool.tile([TOK, O_SHARD], F16)

            # PE p-state warm-up: ~3us of dependency-free dummy matmuls so the
            # clock is ramped before the first weight group lands
            z0 = cpool.tile([128, TOK], F16)
            nc.vector.memset(z0[:], 0.0)
            pw = ps.tile([TOK, TOK], F32, tag="pw", name="pw")
            for _ in range(40):
                nc.tensor.matmul(pw[:], z0[:], z0[:], start=True, stop=True)

            off = 0
            kbase = 0
            for g, sz in enumerate(GROUPS):
                width = sz * O_SHARD
                pool_g = wpool if sz == 4 else wpool2
                wt = pool_g.tile([128, width], F16, tag=f"wt{sz}", name=f"wt{sz}")
                src = wt_d[off:off + 128 * width].rearrange("(p w) -> p w", p=128)
                eng = nc.scalar if g % 2 == 0 else nc.sync
                eng.dma_start(wt[:], src)
                off += 128 * width

                last = g == len(GROUPS) - 1
                if not last:
                    for j in range(sz):
                        k = kbase + j
                        for q in range(4):
                            nc.tensor.matmul(pos[q][:],
                                             xT[:, k * TOK:(k + 1) * TOK],
                                             wt[:, j * O_SHARD + q * OQ:
                                                j * O_SHARD + (q + 1) * OQ],
                                             start=(k == 0), stop=False)
                else:
                    # finish one output quarter at a time; copy + store each
                    # as soon as its accumulation stops
                    for q in range(4):
                        for j in range(sz):
                            k = kbase + j
                            nc.tensor.matmul(pos[q][:],
                                             xT[:, k * TOK:(k + 1) * TOK],
                                             wt[:, j * O_SHARD + q * OQ:
                                                j * O_SHARD + (q + 1) * OQ],
                                             start=False,
                                             stop=(k == K_CHUNKS - 1))
                        nc.scalar.copy(o16[:, q * OQ:(q + 1) * OQ], pos[q][:])
                        nc.sync.dma_start(out_d[:, q * OQ:(q + 1) * OQ],
                                          o16[:, q * OQ:(q + 1) * OQ])
                kbase += sz

    nc.compile()
    return nc


def _get_nc():
    if "nc" not in _CACHE:
        _CACHE["nc"] = _build()
    return _CACHE["nc"]


def _shard(inputs):
    x = np.asarray(inputs["x"], dtype=np.float32)
    codes = np.asarray(inputs["codes"])
    absmax = np.asarray(inputs["absmax"], dtype=np.float32)
    lora_A = np.asarray(inputs["lora_A"], dtype=np.float32)
    lora_B = np.asarray(inputs["lora_B"], dtype=np.float32)

    # effective weights: dequant + LoRA fold (fp32 on host)
    w = NF4[codes].reshape(OUT, N_BLOCKS, BLOCK)
    w *= absmax[:, :, None]
    w = w.reshape(OUT, IN)
    w += LORA_SCALING * (lora_B @ lora_A)
    # [8 cores, IN, O_SHARD] fp16, then per-core per-group swizzle:
    # group block g is [128, GROUPS[g]*O_SHARD] with
    # wTs[g][p, j*O_SHARD+o] = wT[(kbase+j)*128 + p, o]
    wT = np.ascontiguousarray(
        w.T.reshape(IN, N_CORES, O_SHARD).transpose(1, 0, 2)).astype(np.float16)
    parts = []
    kbase = 0
    for sz in GROUPS:
        blk = wT[:, kbase * 128:(kbase + sz) * 128, :]        # [8, sz*128, O]
        blk = blk.reshape(N_CORES, sz, 128, O_SHARD).transpose(0, 2, 1, 3)
        parts.append(blk.reshape(N_CORES, -1))
        kbase += sz
    wTs = np.ascontiguousarray(np.concatenate(parts, axis=1))  # [8, IN*O_SHARD]

    # x -> [128, K_CHUNKS*TOK] fp16 tiled so chunk k occupies cols k*TOK:(k+1)*TOK
    xT = np.ascontiguousarray(x.T).astype(np.float16)          # [IN, TOK]
    xTp = np.ascontiguousarray(
        xT.reshape(K_CHUNKS, 128, TOK).transpose(1, 0, 2).reshape(128, K_CHUNKS * TOK))

    in_maps = []
    for c in range(N_CORES):
        in_maps.append({"xTp": xTp, "wTs": wTs[c]})
    return in_maps


def _run(inputs):
    nc = _get_nc()
    in_maps = _shard(inputs)
    res = run_bass_kernel_spmd(nc, in_maps, core_ids=list(range(N_CORES)))
    out = np.concatenate([res.results[c]["out"] for c in range(N_CORES)], axis=1)
    return np.ascontiguousarray(out.astype(np.float32))


def kernel(**inputs) -> np.ndarray:
    return _run(inputs)
